# revision 1
# baseline (speedup 1.0000x reference)
"""AttentiveVisitPooling Trainium2 kernel (8 NeuronCores, SPMD).

Math: reference computes, for X [N,D], H [N,E] (binary), W,b,v,q,gamma,beta:
    s = tanh(X @ W.T + b + q) @ v                  [N]
    alpha = column-masked softmax of s over nodes  [N, E]
    pooled = alpha.T @ X                           [E, D]
    out = max_E(LayerNorm(pooled))                 [D]

Because the logits of column j are s (shared across columns) masked by H[:, j],
the per-column max-shift cancels:
    alpha[i,j] = H[i,j] * e_i / sum_i H[i,j] * e_i     with e = exp(s)
so with Y = [e*X | e]  (N x (D+1)):
    P = H.T @ Y   gives  P[:, :D] = unnormalized pooled, P[:, D] = denom
and LayerNorm is scale invariant:
    LN(P_raw/denom; eps) == (P_raw - mu_raw) / sqrt(var_raw + eps*denom^2)

Sharding: node axis N split across 8 cores (2500 rows each, zero-padded to
2560; padded rows have H == 0 so they contribute nothing). Each core computes
its s/e slice, its partial P [1024, 257], then an on-device ReduceScatter(add)
gives each core 128 visit rows; each core LayerNorms its rows, applies
gamma/beta, and max-reduces over its visits on device -> [1, 256]. Host
combines the 8 per-core rows with np.maximum.

The end-to-end cost of a kernel() call is dominated by host->device input
transfer (and any per-iteration re-transfer a steady-state bench does), so
the input stream is minimized — 1.12 MB/core vs 9.2 MB/core for the naive
f32 layout:
  * H ships BIT-PACKED (visit j's mask bit = bit j%8 of byte j//8, 328 KB);
    8 DVE shift+and ops expand it to u8 0/1, one copy converts to bf16;
  * x ships as fp8 e4m3 (657 KB) and is upconverted to bf16 by one ACT
    copy; values are ~N(0,1) so e4m3's ~2.4% relative error is safe;
  * params ship once each: W^T bf16, [b+q | ones] bf16, v bf16 [128,2],
    gamma|beta f32 [1,512] partition-broadcast by the DMA itself;
  * both GEMMs run in bf16 (f32 PSUM accumulation), LN in f32; the final
    visit-max runs on device (PE transpose + DVE max-reduce) so the
    output is a single bf16 [1, 256] row.
Measured end-to-end relative error vs the f32 reference: 1.7e-2 (gate
2e-2); the error budget is dominated by the fp8 x in the pooled-value
path and is deterministic for fixed inputs. Drop x to bf16 ([128,NT,DA]
BF16, no cast) if more margin is ever needed.

trn2 backend constraint: Matmult and DMACopy instructions can carry only ONE
attached semaphore wait; _split_multi_waits hoists extra waits onto
single-wait NOPs, and the kernel keeps the baseline's ordering gates (DVE
touch reads, a dummy matmul that really-reads the last Y tile) so the hot
instructions stay single-wait.
"""

import os
import sys

import numpy as np

for _p in ("/opt/trn_rl_repo", "/root/.axon_site/_ro/trn_rl_repo"):
    if _p not in sys.path and os.path.isdir(_p):
        sys.path.append(_p)

import concourse.bass as bass  # noqa: E402
import concourse.tile as tile  # noqa: E402
from concourse import mybir  # noqa: E402
from concourse.bass_utils import run_bass_kernel_spmd  # noqa: E402
from concourse.tile_rust import add_dep_helper  # noqa: E402

N, E, D = 20000, 1024, 256
NCORES = 8
NSH = 2560          # padded per-core node rows (20 x 128)
NT = NSH // 128     # 20 node subtiles
ET = E // 128       # 8 visit subtiles
ER = E // NCORES    # 128 visit rows per core after reduce-scatter
DA = D + 1          # pooled columns + denominator column
E8 = E // 8         # bit-packed visit bytes per node row
LN_EPS = 1e-5

F32 = mybir.dt.float32
BF16 = mybir.dt.bfloat16
F8 = mybir.dt.float8e4

# Toggled by test.py for profiling runs.
PROFILE = False
LAST_EXEC_NS = None
LAST_RESULTS = None

# Timing probes (numerically wrong, timing-only).
SKIP_CC = False       # build without the ReduceScatter
SKIP_COMPUTE = False  # input DMAs only, skip all compute phases

_CACHE = {}


def _build_nc():
    nc = bass.Bass(num_devices=NCORES)

    # x and h arrive host-prepacked in SBUF-native [partition, tile, free]
    # layout so their DMAs are one contiguous run per partition.
    x_d = nc.declare_dram_parameter("x", [128, NT, DA], F8, isOutput=False)
    h_d = nc.declare_dram_parameter("h", [128, NT, E8], mybir.dt.uint8,
                                    isOutput=False)
    wt_d = nc.declare_dram_parameter("wt", [D, D], BF16, isOutput=False)
    # vb row 0: [b+q | zeros], row 1: ones (rank-1 bias rhs over 512 nodes)
    vb_d = nc.declare_dram_parameter("vb", [2, 512], BF16, isOutput=False)
    vv_d = nc.declare_dram_parameter("vv", [128, 2], BF16, isOutput=False)
    gb_d = nc.declare_dram_parameter("gb", [1, 2 * D], F32, isOutput=False)
    out_d = nc.declare_dram_parameter("out_max", [1, D], BF16, isOutput=True)

    _trace_program(nc, x_d, h_d, wt_d, vb_d, vv_d, gb_d, out_d)
    _split_multi_waits(nc)
    return nc


def _trace_program(nc, x_d, h_d, wt_d, vb_d, vv_d, gb_d, out_d):
    with tile.TileContext(nc) as tc:
        with (
            tc.tile_pool(name="consts", bufs=1) as consts,
            tc.tile_pool(name="big", bufs=1) as bigpool,
            tc.tile_pool(name="lnpool", bufs=1) as lnpool,
            tc.tile_pool(name="dram", bufs=1, space="DRAM") as dram,
        ):
            pin = dram.tile([E, DA], F32, tag="pin")
            pout = dram.tile([ER, DA], F32, tag="pout")

            # ---- DMA landings ----
            wt_sb = bigpool.tile([128, 2, D], BF16, tag="wt")
            nc.gpsimd.dma_start(
                out=wt_sb, in_=wt_d.rearrange("(k p) f -> p k f", p=128))
            vb_sb = consts.tile([1, 2, 512], BF16, tag="vb")
            nc.gpsimd.dma_start(out=vb_sb, in_=vb_d[:])
            vv_sb = consts.tile([128, 2], BF16, tag="vv")
            nc.gpsimd.dma_start(out=vv_sb, in_=vv_d[:])
            # gamma|beta: DMA broadcasts the single row across partitions.
            gb_sb = lnpool.tile([128, 2 * D], F32, tag="gb")
            nc.sync.dma_start(out=gb_sb, in_=gb_d[:].to_broadcast((128, 2 * D)))
            gam_sb = gb_sb[:, 0:D]
            bet_sb = gb_sb[:, D:2 * D]

            # x ships fp8 (values ~N(0,1), e4m3 rel err ~2%); one ACT
            # cast rebuilds the bf16 tile everything downstream reads.
            x8_sb = bigpool.tile([128, NT, DA], F8, tag="x8")
            nc.gpsimd.dma_start(out=x8_sb, in_=x_d[:])
            x_sb = bigpool.tile([128, NT, DA], BF16, tag="x")
            for c in range(5):
                nc.scalar.copy(
                    out=x_sb[:, 4 * c:4 * (c + 1), :],
                    in_=x8_sb[:, 4 * c:4 * (c + 1), :])

            # h ships bit-packed (visit j's mask = bit j%8 of byte j//8);
            # 8 DVE shift+and ops expand it to the bf16 tile the GEMM
            # reads. These precede the y build in DVE program order, so
            # the y gate below transitively covers them for PE.
            hb_sb = bigpool.tile([128, NT, E8], mybir.dt.uint8, tag="hb")
            nc.gpsimd.dma_start(out=hb_sb, in_=h_d[:])
            hu_sb = bigpool.tile([128, NT, E], mybir.dt.uint8, tag="hu")
            h_all = bigpool.tile([128, NT, E], BF16, tag="h_all")
            for b in range(8):
                # bitVec ops cannot cast; expand in u8 then convert once.
                nc.vector.tensor_scalar(
                    out=hu_sb[:, :, b:E:8],
                    in0=hb_sb,
                    scalar1=b,
                    scalar2=1,
                    op0=mybir.AluOpType.logical_shift_right,
                    op1=mybir.AluOpType.bitwise_and,
                )
            nc.vector.tensor_copy(h_all, hu_sb)

            e_sb = consts.tile([128, NT], F32, tag="e")
            y_sb = bigpool.tile([128, NT, DA], BF16, tag="y")
            ev_all = consts.tile([128, ET, DA], F32, tag="ev_all")

            # DVE "touch" reads: DVE observes the x/gb DMA lanes on cheap
            # real accesses so downstream DVE ops carry a single wait.
            scratch = consts.tile([128, 4], F32, tag="scratch")
            touch_x = nc.vector.tensor_copy(scratch[:, 0:1], x_sb[:, 0, 0:1])
            touch_g = nc.vector.tensor_copy(scratch[:, 1:2], gam_sb[:, 0:1])
            touch_b = touch_g

            # Rebuild X^T on device: PE transposes of x tiles via identity.
            ident = consts.tile([128, 128], BF16, tag="ident")
            nc.gpsimd.memset(ident, 0.0)
            nc.gpsimd.affine_select(
                out=ident,
                in_=ident,
                compare_op=mybir.AluOpType.not_equal,
                fill=1.0,
                base=0,
                pattern=[[-1, 128]],
                channel_multiplier=1,
            )
            xt_sb = bigpool.tile([128, 2, NSH], BF16, tag="xt")
            with tc.tile_pool(name="tpsum", bufs=2, space="PSUM") as tpsum:
                for t in range(NT):
                    for m in range(2):
                        tp = tpsum.tile([128, 128], BF16, tag="tp")
                        nc.tensor.transpose(
                            tp, x_sb[:, t, m * 128:(m + 1) * 128], ident)
                        nc.scalar.copy(
                            out=xt_sb[:, m, t * 128:(t + 1) * 128], in_=tp)

            if SKIP_COMPUTE:
                # Consume every input stream, write junk output, stop.
                nc.vector.tensor_copy(scratch[:, 2:3], h_all[:, NT - 1, 0:1])
                junk = lnpool.tile([1, D], BF16, tag="junk")
                nc.vector.memset(junk, 0.0)
                nc.sync.dma_start(out=out_d[:], in_=junk)
                return

            # ---- phase 1: s = tanh(X W^T + b + q) @ v ; e = exp(s) ----
            # Transposed layout G^T = W @ X^T; bias as rank-1 matmul so ACT
            # only ever reads PE-written PSUM.
            NCHUNK = NSH // 512  # 5 chunks of 512 nodes
            with (
                tc.tile_pool(name="spsum", bufs=1, space="PSUM") as spsum,
                tc.tile_pool(name="spool", bufs=NCHUNK) as spool,
                # g double-buffered so tanh(c) overlaps the next G matmuls
                tc.tile_pool(name="gpsum", bufs=2, space="PSUM") as gpsum,
            ):
                for c in range(NCHUNK):
                    tt = spool.tile([128, 2, 512], BF16, tag="tt")
                    for m in range(2):
                        g_ps = gpsum.tile([128, 512], F32, tag="g")
                        for k in range(2):
                            nc.tensor.matmul(
                                g_ps,
                                lhsT=wt_sb[:, k, m * 128:(m + 1) * 128],
                                rhs=xt_sb[:, k, c * 512:(c + 1) * 512],
                                start=(k == 0),
                                stop=False,
                            )
                        # += bq[d'] * ones[n]  (rank-1 bias)
                        nc.tensor.matmul(
                            g_ps,
                            lhsT=vb_sb[0:1, 0, m * 128:(m + 1) * 128],
                            rhs=vb_sb[0:1, 1, 0:512],
                            start=False,
                            stop=True,
                        )
                        nc.scalar.activation(
                            out=tt[:, m, :],
                            in_=g_ps,
                            func=mybir.ActivationFunctionType.Tanh,
                        )
                    for j in range(4):
                        t_idx = 4 * c + j
                        s_ps = spsum.tile([128, 1], F32, tag="s")
                        for k in range(2):
                            nc.tensor.matmul(
                                s_ps,
                                lhsT=tt[:, k, j * 128:(j + 1) * 128],
                                rhs=vv_sb[:, k:k + 1],
                                start=(k == 0),
                                stop=(k == 1),
                            )
                        nc.scalar.activation(
                            out=e_sb[:, t_idx:t_idx + 1],
                            in_=s_ps,
                            func=mybir.ActivationFunctionType.Exp,
                        )
                        # Y tile = [e*X | e] immediately after its e: DVE
                        # works during phase 1 instead of serially after it.
                        yi = nc.vector.tensor_scalar_mul(
                            out=y_sb[:, t_idx, :],
                            in0=x_sb[:, t_idx, :],
                            scalar1=e_sb[:, t_idx:t_idx + 1],
                        )
                        if t_idx == 0:
                            add_dep_helper(
                                yi.ins, touch_x.ins, sync=False,
                                reason="order y build after x touch")

                # Dummy matmul really-reads the last y tile: PE observes the
                # whole DVE y-build with one wait, so big-GEMM matmuls only
                # ever wait on their h cast.
                d_ps = spsum.tile([128, 1], F32, tag="dummy")
                ygate = nc.tensor.matmul(
                    d_ps,
                    lhsT=y_sb[:, NT - 1, 0:128],
                    rhs=y_sb[:, NT - 1, 0:1],
                    start=True,
                    stop=True,
                )

                # ---- phase 2: partial P = H^T @ Y, e-outer in PSUM bank
                # groups of 5 + 3 (spsum stays alive -> disjoint banks).
                pin_v = pin.rearrange("(e8 p) d -> p e8 d", p=128)
                with tc.tile_pool(name="bpsum", bufs=1, space="PSUM") as bpsum:
                    for grp, lo, hi in ((0, 0, 4), (1, 4, 8)):
                        pps = [
                            bpsum.tile([128, DA], F32, tag=f"pp{gi}",
                                       name=f"pp{gi}")
                            for gi in range(hi - lo)
                        ]
                        for t in range(NT):
                            for gi, e8 in enumerate(range(lo, hi)):
                                mm = nc.tensor.matmul(
                                    pps[gi],
                                    lhsT=h_all[:, t, e8 * 128:(e8 + 1) * 128],
                                    rhs=y_sb[:, t, :],
                                    start=(t == 0),
                                    stop=(t == NT - 1),
                                )
                                if t == 0:
                                    add_dep_helper(
                                        mm.ins, ygate.ins, sync=False,
                                        reason="order big GEMM after y gate")
                        for gi, e8 in enumerate(range(lo, hi)):
                            nc.scalar.copy(out=ev_all[:, e8, :], in_=pps[gi])
                        # per-group evacuation DMA (overlaps group 2's
                        # GEMM with group 1's writeback)
                        nc.gpsimd.dma_start(
                            out=pin_v[:, lo:hi, :], in_=ev_all[:, lo:hi, :])

            # ---- phase 3: reduce-scatter partials across the 8 cores ----
            # (a split two-RS variant simulated slower: per-collective
            # overhead exceeds the overlap gain)
            if not SKIP_CC:
                nc.gpsimd.collective_compute(
                    "ReduceScatter",
                    mybir.AluOpType.add,
                    replica_groups=[list(range(NCORES))],
                    ins=[pin[:].opt()],
                    outs=[pout[:].opt()],
                )
            else:
                nc.gpsimd.dma_start(out=pout[:], in_=pin[0:ER, :])

            # ---- phase 4: LayerNorm rows + gamma/beta + max over visits ----
            if True:
                rs = lnpool.tile([128, DA], F32, tag="rs")
                nc.sync.dma_start(out=rs, in_=pout[:])

                stats = lnpool.tile([128, 6], F32, tag="stats")
                nc.vector.bn_stats(out=stats, in_=rs[:, 0:D])
                mv = lnpool.tile([128, 2], F32, tag="mv")
                nc.vector.bn_aggr(out=mv, in_=stats)

                # tvar = var + eps * denom^2  (LayerNorm scale invariance)
                den2 = lnpool.tile([128, 1], F32, tag="den2")
                nc.vector.tensor_mul(out=den2, in0=rs[:, D:DA], in1=rs[:, D:DA])
                tvar = lnpool.tile([128, 1], F32, tag="tvar")
                nc.vector.tensor_scalar(
                    out=tvar,
                    in0=den2,
                    scalar1=LN_EPS,
                    scalar2=mv[:, 1:2],
                    op0=mybir.AluOpType.mult,
                    op1=mybir.AluOpType.add,
                )
                nc.vector.tensor_scalar_max(out=tvar, in0=tvar, scalar1=1e-38)
                rstd = lnpool.tile([128, 1], F32, tag="rstd")
                nc.scalar.activation(
                    out=rstd, in_=tvar, func=mybir.ActivationFunctionType.Sqrt
                )
                nc.vector.reciprocal(out=rstd, in_=rstd)

                z = lnpool.tile([128, D], F32, tag="z")
                nc.vector.tensor_scalar(
                    out=z,
                    in0=rs[:, 0:D],
                    scalar1=mv[:, 0:1],
                    scalar2=rstd,
                    op0=mybir.AluOpType.subtract,
                    op1=mybir.AluOpType.mult,
                )
                vn = lnpool.tile([128, D], F32, tag="vn")
                vm = nc.vector.tensor_mul(out=vn, in0=z, in1=gam_sb)
                add_dep_helper(vm.ins, touch_g.ins, sync=False,
                               reason="order after gamma touch")
                va = nc.vector.tensor_add(out=vn, in0=vn, in1=bet_sb)
                add_dep_helper(va.ins, touch_b.ins, sync=False,
                               reason="order after beta touch")

                # Visit-axis max on device: cast to bf16, PE-transpose
                # the two 128-wide halves (d onto partitions), DVE max-reduce
                # over visits, and ship a single [1, D] bf16 row.
                vnb = lnpool.tile([128, D], BF16, tag="vnb")
                nc.vector.tensor_copy(vnb, vn)
                vt = lnpool.tile([128, 2, 128], BF16, tag="vt")
                with tc.tile_pool(name="vpsum", bufs=2, space="PSUM") as vpsum:
                    for m in range(2):
                        tpv = vpsum.tile([128, 128], BF16, tag="tpv")
                        nc.tensor.transpose(
                            tpv, vnb[:, m * 128:(m + 1) * 128], ident)
                        nc.scalar.copy(out=vt[:, m, :], in_=tpv)
                vmax2 = lnpool.tile([128, 2], BF16, tag="vmax2")
                nc.vector.tensor_reduce(
                    out=vmax2, in_=vt, axis=mybir.AxisListType.X,
                    op=mybir.AluOpType.max)
                nc.sync.dma_start(
                    out=out_d[0, :].rearrange("(m p) -> p m", p=128),
                    in_=vmax2)

                # Tail re-read of pout on the SP queue: lets the kernel-tail
                # drain elide the Collectives semaphore wait.
                tail = lnpool.tile([128, 1], F32, tag="tail")
                nc.sync.dma_start(out=tail[0:1, 0:1], in_=pout[0:1, 0:1])


def _split_multi_waits(nc):
    """Walrus codegen accepts at most one attached semaphore wait per
    instruction; hoist extra waits onto single-wait NOPs just before."""
    for blk in nc.m.functions[0].blocks:
        insts = list(blk.instructions)
        out = []
        changed = False
        for inst in insts:
            si = inst.sync_info
            if si is not None and si.on_wait is not None and len(si.on_wait) > 1:
                waits = list(si.on_wait)
                for w in waits[:-1]:
                    nop = mybir.InstNoOp(
                        name=f"I-wsplit-{nc.next_id()}",
                        sync_info=mybir.SyncInfo(on_wait=[w], on_update=[]),
                        bass_nofuse=True,
                        engine=inst.engine,
                    )
                    out.append(nop)
                inst.sync_info = mybir.SyncInfo(
                    on_wait=[waits[-1]], on_update=list(si.on_update or [])
                )
                changed = True
            out.append(inst)
        if changed:
            blk.instructions = out


def _get_nc():
    if "nc" not in _CACHE:
        _CACHE["nc"] = _build_nc()
    return _CACHE["nc"]


def prepare_in_maps(node_embeddings, H, W, b, v, q, ln_gamma, ln_beta):
    import ml_dtypes

    bf16 = ml_dtypes.bfloat16
    f8 = mybir.dt.np(F8)
    x_full = np.asarray(node_embeddings, dtype=np.float32)
    h_full = np.asarray(H, dtype=np.float32)
    wt = np.ascontiguousarray(
        np.asarray(W, dtype=np.float32).T.astype(bf16))  # wt[d,d']=W[d',d]
    bq = (np.asarray(b, dtype=np.float32) + np.asarray(q, dtype=np.float32))
    v_np = np.asarray(v, dtype=np.float32)
    gam = np.asarray(ln_gamma, dtype=np.float32)
    bet = np.asarray(ln_beta, dtype=np.float32)

    # h is binary; pack 8 visit columns per byte (little bit order).
    h_bits = np.packbits(h_full != 0, axis=1, bitorder="little")  # [N, E8]

    vb = np.zeros((2, 512), np.float32)
    vb[0, :D] = bq
    vb[1, :] = 1.0
    vb = vb.astype(bf16)
    vv = np.ascontiguousarray(v_np.reshape(2, 128).T.astype(bf16))
    gb = np.concatenate([gam, bet]).reshape(1, 2 * D).astype(np.float32)

    nsh_rows = N // NCORES  # 2500
    in_maps = []
    for k in range(NCORES):
        r0 = k * nsh_rows
        x_k = np.zeros((NSH, DA), f8)
        x_k[:nsh_rows, :D] = x_full[r0:r0 + nsh_rows].astype(f8)
        x_k[:, D] = np.float32(1.0)
        h_k = np.zeros((NSH, E8), np.uint8)
        h_k[:nsh_rows] = h_bits[r0:r0 + nsh_rows]

        # Prepack to SBUF-native [partition, tile, free] layout.
        xp = np.ascontiguousarray(
            x_k.reshape(NT, 128, DA).transpose(1, 0, 2))
        hp = np.ascontiguousarray(
            h_k.reshape(NT, 128, E8).transpose(1, 0, 2))
        in_maps.append(
            {"x": xp, "h": hp, "wt": wt, "vb": vb, "vv": vv, "gb": gb})
    return in_maps


def kernel(node_embeddings, H, W, b, v, q, ln_gamma, ln_beta):
    global LAST_EXEC_NS, LAST_RESULTS

    in_maps = prepare_in_maps(
        node_embeddings, H, W, b, v, q, ln_gamma, ln_beta)
    nc = _get_nc()
    res = run_bass_kernel_spmd(
        nc, in_maps, core_ids=list(range(NCORES)), trace=PROFILE
    )
    LAST_EXEC_NS = res.exec_time_ns
    LAST_RESULTS = res
    outs = [
        res.results[k]["out_max"][0].astype(np.float32)
        for k in range(NCORES)
    ]
    return np.maximum.reduce(outs).astype(np.float32)



# revision 59
# speedup vs baseline: 1.7905x; 1.7905x over previous
"""AttentiveVisitPooling Trainium2 kernel (8 NeuronCores, SPMD).

Math: reference computes, for X [N,D], H [N,E] (binary), W,b,v,q,gamma,beta:
    s = tanh(X @ W.T + b + q) @ v                  [N]
    alpha = column-masked softmax of s over nodes  [N, E]
    pooled = alpha.T @ X                           [E, D]
    out = max_E(LayerNorm(pooled))                 [D]

Per-column max-shift cancels (logits of column j are s masked by H[:, j]):
    alpha[i,j] = H[i,j] e_i / sum_i H[i,j] e_i     with e = exp(s)
so with Y = [e*X | e]  (N x (D+1)):
    P = H.T @ Y   gives  P[:, :D] = unnormalized pooled, P[:, D] = denom
and LayerNorm is scale invariant:
    LN(P_raw/denom; eps) == (P_raw - mu_raw) / sqrt(var_raw + eps*denom^2)
A *uniform* scale c on a whole P row cancels entirely in that expression, so
Y can be stored as c*Y for any c>0; we use c=1/8 (folded into exp's bias as
-ln 8) to keep fp8 Y far from e4m3 saturation.

Sharding: node axis N split across 8 cores (2500 rows each, zero-padded to
2560; padded rows have H == 0 so they contribute nothing). Each core
computes its partial P [1024, 257] in bf16, an on-device ReduceScatter(add)
gives each core 128 visit rows; each core LayerNorms its rows, applies
gamma/beta, max-reduces over its visits on device -> [1, 256] bf16. Host
combines the 8 per-core rows with np.maximum.

Device-time structure (cost-model sim):
  * x ships bf16 [128, NT, 257] SBUF-native; ones in col 256 make the
    denominator fall out of the same GEMM.
  * h ships BIT-PLANE packed u8 [128, NT, 128]: byte i of a row holds
    visits {blk*128+i : blk in 0..8} at bit blk. One two-op DVE
    tensor_scalar per visit block (mod 2^(blk+1), is_ge 2^blk) expands
    bits directly to fp8 1.0/0.0 - single pass, no u8 intermediate.
  * phase 1 (s = tanh(X W^T + b + q) @ v) in bf16: W X^T via PE with the
    b+q bias applied by the tanh activation's per-partition bias operand
    (no rank-1 bias matmuls); exp batched 4 columns per ACT instruction
    with bias -ln 8; Y tiles built by DVE right after each e chunk.
  * phase 2 (P = H^T Y) in fp8 DoubleRow perf mode: consecutive node
    subtile pairs fuse into one matmul (lhsT [128,2,128], rhs [128,2,257]).
  * ReduceScatter payload bf16 (error-neutral, measured).
  * LN tail fused: bn_stats/bn_aggr, den^2*eps+var in one tensor_scalar,
    (raw-mu)*rstd in one tensor_scalar, visit-max via PE transpose + DVE
    reduce straight out of PSUM.

trn2 backend constraint: Matmult and DMACopy instructions can carry only
ONE attached semaphore wait; _split_multi_waits hoists extras onto
single-wait NOPs.
"""

import math
import os
import sys

import numpy as np

for _p in ("/opt/trn_rl_repo", "/root/.axon_site/_ro/trn_rl_repo"):
    if _p not in sys.path and os.path.isdir(_p):
        sys.path.append(_p)

import concourse.bass as bass  # noqa: E402
import concourse.tile as tile  # noqa: E402
from concourse import mybir  # noqa: E402
from concourse.bass_utils import run_bass_kernel_spmd  # noqa: E402
from concourse.tile_rust import add_dep_helper  # noqa: E402

N, E, D = 20000, 1024, 256
NCORES = 8
NSH = 2560          # padded per-core node rows (20 x 128)
NT = NSH // 128     # 20 node subtiles
NPAIR = NT // 2     # 10 DoubleRow node-subtile pairs
ET = E // 128       # 8 visit blocks
ER = E // NCORES    # 128 visit rows per core after reduce-scatter
DA = D + 1          # pooled columns + denominator column
DAP = 272           # DA padded to 16B so DoubleRow's moving step is legal
LN_EPS = 1e-5
YC_BIAS = -math.log(8.0)   # exp bias: e_sb holds e/8

F32 = mybir.dt.float32
BF16 = mybir.dt.bfloat16
F8 = mybir.dt.float8e4

# Toggled by test.py for profiling runs.
PROFILE = False
LAST_EXEC_NS = None
LAST_RESULTS = None

# Timing probes (numerically wrong, timing-only).
SKIP_CC = False       # build without the ReduceScatter
SKIP_COMPUTE = False  # input DMAs only, skip all compute phases
SPLIT_WAITS = True    # walrus needs single-wait insts; CoreSim chokes on NOPs

# Program specialization: gamma==1 and beta==0 (true for the reference's
# setup_inputs) lets the tail skip the two gamma/beta tensor ops. kernel()
# picks the right build per call; both variants are cached.
LN_IDENTITY = True

# Phase-2 GEMM flavor: "dr" = fp8 DoubleRow (paired node subtiles),
# "fp8" = plain fp8 matmuls per subtile (isolates DoubleRow layout bugs).
PH2 = "dr"

# Debug build: dump intermediates (e, h8, y8, pin) as extra outputs.
DEBUG = False

_CACHE = {}


def _build_nc():
    nc = bass.Bass(num_devices=NCORES)

    # x arrives node-row-major so the XBAR DMA transpose can produce X^T
    # directly; h arrives host-prepacked in SBUF-native layout.
    x_d = nc.declare_dram_parameter("x", [NSH, DAP], BF16, isOutput=False)
    xt_d = nc.declare_dram_parameter("xt", [2, 128, NSH], BF16,
                                     isOutput=False)
    h_d = nc.declare_dram_parameter("h", [128, NT, 128], mybir.dt.uint8,
                                    isOutput=False)
    wt_d = nc.declare_dram_parameter("wt", [128, 2, D], BF16, isOutput=False)
    pv_d = nc.declare_dram_parameter("pv", [128, 4], BF16, isOutput=False)
    gb_d = nc.declare_dram_parameter("gb", [1, 2 * D], F32, isOutput=False)
    out_d = nc.declare_dram_parameter("out_max", [1, D], BF16, isOutput=True)
    dbg = None
    if DEBUG:
        dbg = {
            "dbg_e": nc.declare_dram_parameter(
                "dbg_e", [128, NT], F32, isOutput=True),
            "dbg_h8": nc.declare_dram_parameter(
                "dbg_h8", [128, NT, E], mybir.dt.uint8, isOutput=True),
            "dbg_y8": nc.declare_dram_parameter(
                "dbg_y8", [128, NT, DAP], mybir.dt.uint8, isOutput=True),
            "dbg_pin": nc.declare_dram_parameter(
                "dbg_pin", [128, ET, DA], BF16, isOutput=True),
        }

    _trace_program(nc, x_d, xt_d, h_d, wt_d, pv_d, gb_d, out_d, dbg)
    if SPLIT_WAITS:
        _split_multi_waits(nc)
    return nc


def _trace_program(nc, x_d, xt_d, h_d, wt_d, pv_d, gb_d, out_d, dbg=None):
    with tile.TileContext(nc) as tc:
        with (
            tc.tile_pool(name="consts", bufs=1) as consts,
            tc.tile_pool(name="big", bufs=1) as bigpool,
            tc.tile_pool(name="lnpool", bufs=1) as lnpool,
            tc.tile_pool(name="dram", bufs=1, space="DRAM") as dram,
        ):
            pin = dram.tile([E, DA], BF16, tag="pin")
            pout = dram.tile([ER, DA], BF16, tag="pout")

            # ---- DMA landings ----
            # Emission order = rough chaining order; phase-1's gating params
            # (wt on SP, bqp/vv on ACT behind the xt transpose) land first;
            # h bits head the Pool queue for the DVE bit-expand long pole.
            hb_sb = bigpool.tile([128, NT, 128], mybir.dt.uint8, tag="hb")
            nc.gpsimd.dma_start(out=hb_sb, in_=h_d[:])
            wt_sb = bigpool.tile([128, 2, D], BF16, tag="wt")
            nc.sync.dma_start(out=wt_sb, in_=wt_d[:])
            # bqp|vv merged into one bf16 [128, 4] param DMA (cols 0-1 =
            # b+q halves for the tanh bias, cols 2-3 = v halves).
            pv_sb = consts.tile([128, 4], BF16, tag="pv")
            nc.scalar.dma_start(out=pv_sb, in_=pv_d[:])
            bqp_sb = pv_sb[:, 0:2]
            vv_sb = pv_sb[:, 2:4]

            # X^T ships host-pretransposed, one 128-row half per HWDGE
            # queue, in SEPARATE tiles so readers of half 0 don't wait for
            # half 1. (An XBAR DMA transpose raced on real HW: its
            # completion semaphore fires before all tiles land.)
            xt0_sb = bigpool.tile([128, NSH], BF16, tag="xt0")
            xt1_sb = bigpool.tile([128, NSH], BF16, tag="xt1")
            nc.scalar.dma_start(out=xt0_sb, in_=xt_d[0])
            nc.sync.dma_start(out=xt1_sb, in_=xt_d[1])
            xt_k = (xt0_sb, xt1_sb)

            # Warm the exp/tanh activation table on the idle ACT engine so
            # phase 1's first tanh skips the 1.28us table load.
            ew = consts.tile([128, 1], F32, tag="ew")
            nc.vector.memset(ew, 0.0)
            nc.scalar.activation(
                out=ew, in_=ew, func=mybir.ActivationFunctionType.Exp)

            # ident (tail's visit-max transposes) on the Pool ALU, behind
            # the h DMA — not needed until the very end.
            ident = consts.tile([128, 128], BF16, tag="ident")
            nc.gpsimd.memset(ident, 0.0)
            nc.gpsimd.affine_select(
                out=ident,
                in_=ident,
                compare_op=mybir.AluOpType.not_equal,
                fill=1.0,
                base=0,
                pattern=[[-1, 128]],
                channel_multiplier=1,
            )
            # PE clock warmup: ~3us of dummy matmuls on a zeroed scratch so
            # the first real G matmul runs at full clock instead of the
            # 0.65GHz cold pstate. Scratch comes from DVE at t~0.
            wsc = consts.tile([128, 512], BF16, tag="wsc")
            nc.vector.memset(wsc, 0.0)
            with tc.tile_pool(name="wpsum", bufs=1, space="PSUM") as wpsum:
                wp = wpsum.tile([128, 512], F32, tag="wp")
                for i in range(9):
                    nc.tensor.matmul(
                        wp, lhsT=wsc[:, 0:128], rhs=wsc,
                        start=True, stop=True)

            # x in node-subtile layout for the Y build, chunked on SP.
            x_sb = bigpool.tile([128, NT, DAP], BF16, tag="x")
            x_dv = x_d.rearrange("(t p) d -> p t d", p=128)
            for c in range(5):
                nc.sync.dma_start(
                    out=x_sb[:, 4 * c:4 * (c + 1), :],
                    in_=x_dv[:, 4 * c:4 * (c + 1), :])
            # gamma|beta: DMA broadcasts the single row across partitions.
            gb_sb = lnpool.tile([128, 2 * D], F32, tag="gb")
            if not LN_IDENTITY:
                nc.sync.dma_start(
                    out=gb_sb, in_=gb_d[:].to_broadcast((128, 2 * D)))
            gam_sb = gb_sb[:, 0:D]
            bet_sb = gb_sb[:, D:2 * D]

            # ---- h bit-expand: one shift+and bitvec pass per visit block --
            # byte value B holds visit blk at bit blk. Shifting that bit to
            # position 3 and masking with 0x08 yields the fp8e4m3 BIT
            # PATTERN for 2^-6 (a normal value) - a uniform scale on the
            # whole P row, which LayerNorm's scale invariance cancels. The
            # pass runs on u16 word pairs (mask 0x0808: shifts <= 3 never
            # cross a byte boundary) so every operand is 2-byte and DVE's
            # fast mode applies. Pool can't host TensorScalar on real HW,
            # so all blocks run on DVE.
            h8_sb = bigpool.tile([128, NT, E], F8, tag="h8")
            hb16 = hb_sb.bitcast(mybir.dt.uint16)

            def _h_expand(blk):
                if blk <= 3:
                    op0, s1 = mybir.AluOpType.logical_shift_left, 3 - blk
                else:
                    op0, s1 = mybir.AluOpType.logical_shift_right, blk - 3
                nc.vector.tensor_scalar(
                    out=h8_sb[:, :, blk * 128:(blk + 1) * 128].bitcast(
                        mybir.dt.uint16),
                    in0=hb16,
                    scalar1=s1,
                    scalar2=0x0808,
                    op0=op0,
                    op1=mybir.AluOpType.bitwise_and,
                )

            for blk in range(ET):
                _h_expand(blk)

            if SKIP_COMPUTE:
                junk = lnpool.tile([1, D], BF16, tag="junk")
                nc.vector.memset(junk, 0.0)
                nc.vector.tensor_copy(junk[0:1, 0:1], h8_sb[0:1, NT - 1, 0:1])
                nc.vector.tensor_copy(junk[0:1, 1:2], xt_sb[0:1, 1, 0:1])
                nc.vector.tensor_copy(junk[0:1, 2:3], x_sb[0:1, NT - 1, 0:1])
                nc.sync.dma_start(out=out_d[:], in_=junk)
                return

            # ---- phase 1: s = tanh(X W^T + b + q) @ v ; e = exp(s)/8 ----
            # Transposed layout G^T = W @ X^T; b+q lands via the tanh
            # activation's per-partition bias operand.
            e_sb = consts.tile([128, NT], F32, tag="e")
            ycb = consts.tile([128, 1], F32, tag="ycb")
            nc.gpsimd.memset(ycb, YC_BIAS)
            y8_sb = bigpool.tile([128, NT, DAP], F8, tag="y8")
            pin_sb = bigpool.tile([128, ET, DA], BF16, tag="pin_sb")
            pin_v = pin.rearrange("(e8 p) d -> p e8 d", p=128)
            NCHUNK = NSH // 512  # 5 chunks of 512 nodes
            NA = 5  # visit blocks accumulated in-loop (PSUM: 2+1+NA <= 8)

            def _dr(pp, p, blk, start, stop):
                if PH2 == "dr":
                    nc.tensor.matmul(
                        pp,
                        lhsT=h8_sb[:, 2 * p:2 * p + 2,
                                   blk * 128:(blk + 1) * 128],
                        rhs=y8_sb[:, 2 * p:2 * p + 2, :],
                        start=start,
                        stop=stop,
                        perf_mode=mybir.MatmulPerfMode.DoubleRow,
                    )
                else:
                    for i in range(2):
                        nc.tensor.matmul(
                            pp,
                            lhsT=h8_sb[:, 2 * p + i,
                                       blk * 128:(blk + 1) * 128],
                            rhs=y8_sb[:, 2 * p + i, :],
                            start=start and i == 0,
                            stop=stop and i == 1,
                        )

            with tc.tile_pool(name="bpsumA", bufs=1, space="PSUM") as bpsumA:
                ppsA = [
                    bpsumA.tile([128, DAP], F32, tag=f"pp{blk}",
                                name=f"pp{blk}")
                    for blk in range(NA)
                ]
                with (
                    tc.tile_pool(name="spsum", bufs=1, space="PSUM") as spsum,
                    tc.tile_pool(name="spool", bufs=2) as spool,
                    tc.tile_pool(name="gpsum", bufs=2, space="PSUM") as gpsum,
                ):
                    for c in range(NCHUNK):
                        tt = spool.tile([128, 2, 512], BF16, tag="tt")
                        for m in range(2):
                            g_ps = gpsum.tile([128, 512], F32, tag="g")
                            for k in range(2):
                                nc.tensor.matmul(
                                    g_ps,
                                    lhsT=wt_sb[:, k, m * 128:(m + 1) * 128],
                                    rhs=xt_k[k][:, c * 512:(c + 1) * 512],
                                    start=(k == 0),
                                    stop=(k == 1),
                                )
                            nc.scalar.activation(
                                out=tt[:, m, :],
                                in_=g_ps,
                                func=mybir.ActivationFunctionType.Tanh,
                                bias=bqp_sb[:, m:m + 1],
                            )
                        s_ps = spsum.tile([128, 4], F32, tag="s")
                        for j in range(4):
                            for m in range(2):
                                nc.tensor.matmul(
                                    s_ps[:, j:j + 1],
                                    lhsT=tt[:, m, j * 128:(j + 1) * 128],
                                    rhs=vv_sb[:, m:m + 1],
                                    start=(m == 0),
                                    stop=(m == 1),
                                )
                        exp_i = nc.scalar.activation(
                            out=e_sb[:, 4 * c:4 * (c + 1)],
                            in_=s_ps,
                            func=mybir.ActivationFunctionType.Exp,
                            bias=ycb[:, 0:1],
                        )
                        for j in range(4):
                            t_idx = 4 * c + j
                            nc.vector.tensor_scalar_mul(
                                out=y8_sb[:, t_idx, :],
                                in0=x_sb[:, t_idx, :],
                                scalar1=e_sb[:, t_idx:t_idx + 1],
                            )
                        # A-group DoubleRows for the PREVIOUS chunk's pairs:
                        # overlaps phase-2 with phase 1 on PE idle gaps.
                        if c >= 1:
                            for p in (2 * c - 2, 2 * c - 1):
                                for blk in range(NA):
                                    _dr(ppsA[blk], p, blk,
                                        start=(p == 0), stop=False)

                # phase-1 PSUM freed; finish A pairs 8,9 and all of B.
                with tc.tile_pool(name="bpsumB", bufs=1,
                                  space="PSUM") as bpsumB:
                    ppsB = [
                        bpsumB.tile([128, DAP], F32, tag=f"pp{blk}",
                                    name=f"pp{blk}")
                        for blk in range(NA, ET)
                    ]
                    for p in (NPAIR - 2, NPAIR - 1):
                        for blk in range(NA):
                            _dr(ppsA[blk], p, blk,
                                start=False, stop=(p == NPAIR - 1))
                    for p in range(NPAIR):
                        for blk in range(NA, ET):
                            _dr(ppsB[blk - NA], p, blk,
                                start=(p == 0), stop=(p == NPAIR - 1))

                    # evacuate alternating DVE/ACT so copies pipeline 2-wide
                    for blk in range(ET):
                        pp = ppsA[blk] if blk < NA else ppsB[blk - NA]
                        if blk % 2 == 0:
                            nc.vector.tensor_copy(pin_sb[:, blk, :], pp[:, 0:DA])
                        else:
                            nc.scalar.copy(out=pin_sb[:, blk, :], in_=pp[:, 0:DA])
                        if blk == NA - 1:
                            nc.gpsimd.dma_start(
                                out=pin_v[:, 0:NA, :], in_=pin_sb[:, 0:NA, :])
                    nc.gpsimd.dma_start(
                        out=pin_v[:, NA:ET, :], in_=pin_sb[:, NA:ET, :])

            if dbg is not None:
                nc.sync.dma_start(out=dbg["dbg_e"][:], in_=e_sb)
                nc.sync.dma_start(
                    out=dbg["dbg_h8"][:],
                    in_=h8_sb.bitcast(mybir.dt.uint8))
                nc.sync.dma_start(
                    out=dbg["dbg_y8"][:],
                    in_=y8_sb.bitcast(mybir.dt.uint8))
                nc.sync.dma_start(out=dbg["dbg_pin"][:], in_=pin_sb)

            # Preload the sqrt activation table while ACT idles during the
            # collective — the LN tail's Sqrt then skips the 1.28us load.
            # The explicit dep keeps the scheduler from hoisting the table
            # switch into the middle of the phase-1 tanh/exp chain.
            sq_warm = lnpool.tile([128, 1], F32, tag="sq_warm")
            nc.gpsimd.memset(sq_warm, 1.0)
            sqw = nc.scalar.activation(
                out=sq_warm, in_=sq_warm,
                func=mybir.ActivationFunctionType.Sqrt)
            add_dep_helper(sqw.ins, exp_i.ins, sync=False,
                           reason="sqrt table load only after last exp")

            # ---- phase 3: reduce-scatter partials across the 8 cores ----
            if not SKIP_CC:
                nc.gpsimd.collective_compute(
                    "ReduceScatter",
                    mybir.AluOpType.add,
                    replica_groups=[list(range(NCORES))],
                    ins=[pin[:].opt()],
                    outs=[pout[:].opt()],
                )
            else:
                nc.gpsimd.dma_start(out=pout[:], in_=pin[0:ER, :])

            # ---- phase 4: LayerNorm rows + gamma/beta + max over visits ----
            rs = lnpool.tile([128, DA], BF16, tag="rs")
            nc.sync.dma_start(out=rs, in_=pout[:])

            stats = lnpool.tile([128, 6], F32, tag="stats")
            nc.vector.bn_stats(out=stats, in_=rs[:, 0:D])
            mv = lnpool.tile([128, 2], F32, tag="mv")
            nc.vector.bn_aggr(out=mv, in_=stats)

            # tvar = var + eps * denom^2  (LayerNorm scale invariance)
            den_f = lnpool.tile([128, 1], F32, tag="den_f")
            nc.vector.tensor_copy(den_f, rs[:, D:DA])
            den2e = lnpool.tile([128, 1], F32, tag="den2e")
            nc.vector.tensor_scalar(
                out=den2e,
                in0=den_f,
                scalar1=den_f,
                scalar2=LN_EPS,
                op0=mybir.AluOpType.mult,
                op1=mybir.AluOpType.mult,
            )
            tvar = lnpool.tile([128, 1], F32, tag="tvar")
            nc.vector.tensor_scalar(
                out=tvar,
                in0=den2e,
                scalar1=mv[:, 1:2],
                scalar2=1e-38,
                op0=mybir.AluOpType.add,
                op1=mybir.AluOpType.max,
            )
            rstd = lnpool.tile([128, 1], F32, tag="rstd")
            nc.scalar.activation(
                out=rstd, in_=tvar, func=mybir.ActivationFunctionType.Sqrt
            )
            nc.vector.reciprocal(out=rstd, in_=rstd)

            vnb = lnpool.tile([128, D], BF16, tag="vnb")
            if LN_IDENTITY:
                nc.vector.tensor_scalar(
                    out=vnb,
                    in0=rs[:, 0:D],
                    scalar1=mv[:, 0:1],
                    scalar2=rstd,
                    op0=mybir.AluOpType.subtract,
                    op1=mybir.AluOpType.mult,
                )
            else:
                z = lnpool.tile([128, D], F32, tag="z")
                nc.vector.tensor_scalar(
                    out=z,
                    in0=rs[:, 0:D],
                    scalar1=mv[:, 0:1],
                    scalar2=rstd,
                    op0=mybir.AluOpType.subtract,
                    op1=mybir.AluOpType.mult,
                )
                vn = lnpool.tile([128, D], F32, tag="vn")
                nc.vector.tensor_mul(out=vn, in0=z, in1=gam_sb)
                nc.vector.tensor_add(out=vnb, in0=vn, in1=bet_sb)

            # Visit-axis max: PE-transpose the two 128-wide halves (d onto
            # partitions), DVE max-reduce straight out of PSUM.
            with tc.tile_pool(name="vpsum", bufs=1, space="PSUM") as vpsum:
                vt = vpsum.tile([128, 2, 128], BF16, tag="vt")
                for m in range(2):
                    nc.tensor.transpose(
                        vt[:, m, :], vnb[:, m * 128:(m + 1) * 128], ident)
                vmax2 = lnpool.tile([128, 2], BF16, tag="vmax2")
                nc.vector.tensor_reduce(
                    out=vmax2, in_=vt, axis=mybir.AxisListType.X,
                    op=mybir.AluOpType.max)
            nc.sync.dma_start(
                out=out_d[0, :].rearrange("(m p) -> p m", p=128),
                in_=vmax2)

            # Tail re-read of pout on the SP queue: lets the kernel-tail
            # drain elide the Collectives semaphore wait.
            tail = lnpool.tile([128, 1], BF16, tag="tail")
            nc.sync.dma_start(out=tail[0:1, 0:1], in_=pout[0:1, 0:1])


def _split_multi_waits(nc):
    """Walrus codegen accepts at most one attached semaphore wait per
    instruction; hoist extra waits onto single-wait NOPs just before."""
    for blk in nc.m.functions[0].blocks:
        insts = list(blk.instructions)
        out = []
        changed = False
        for inst in insts:
            si = inst.sync_info
            if si is not None and si.on_wait is not None and len(si.on_wait) > 1:
                waits = list(si.on_wait)
                for w in waits[:-1]:
                    nop = mybir.InstNoOp(
                        name=f"I-wsplit-{nc.next_id()}",
                        sync_info=mybir.SyncInfo(on_wait=[w], on_update=[]),
                        bass_nofuse=True,
                        engine=inst.engine,
                    )
                    out.append(nop)
                inst.sync_info = mybir.SyncInfo(
                    on_wait=[waits[-1]], on_update=list(si.on_update or [])
                )
                changed = True
            out.append(inst)
        if changed:
            blk.instructions = out


def _get_nc():
    key = ("nc", LN_IDENTITY)
    if key not in _CACHE:
        _CACHE[key] = _build_nc()
    return _CACHE[key]


def prepare_in_maps(node_embeddings, H, W, b, v, q, ln_gamma, ln_beta):
    import ml_dtypes

    bf16 = ml_dtypes.bfloat16
    x_full = np.asarray(node_embeddings, dtype=np.float32)
    h_full = np.asarray(H, dtype=np.float32)
    wt = np.ascontiguousarray(
        np.asarray(W, dtype=np.float32).T.reshape(2, 128, D)
        .transpose(1, 0, 2).astype(bf16))          # wt[p,k,f] = W[f, k*128+p]
    bq = (np.asarray(b, dtype=np.float32) + np.asarray(q, dtype=np.float32))
    pv = np.concatenate(
        [bq.reshape(2, 128).T, np.asarray(v, np.float32).reshape(2, 128).T],
        axis=1).astype(bf16)                       # [128, 4] = bqp | vv
    gam = np.asarray(ln_gamma, dtype=np.float32)
    bet = np.asarray(ln_beta, dtype=np.float32)
    gb = np.concatenate([gam, bet]).reshape(1, 2 * D).astype(np.float32)

    # h: bit-plane pack. byte[n, i] holds visits {blk*128 + i} at bit blk.
    hv = (h_full != 0).reshape(N, ET, 128)                  # [N, blk, i]
    h_bits = np.packbits(
        hv.transpose(0, 2, 1), axis=2, bitorder="little")   # [N, 128, 1]
    h_bits = np.ascontiguousarray(h_bits[:, :, 0])          # [N, 128]

    nsh_rows = N // NCORES  # 2500
    in_maps = []
    for k in range(NCORES):
        r0 = k * nsh_rows
        x_k = np.zeros((NSH, DAP), bf16)
        x_k[:nsh_rows, :D] = x_full[r0:r0 + nsh_rows].astype(bf16)
        x_k[:, D] = np.float32(1.0)
        h_k = np.zeros((NSH, 128), np.uint8)
        h_k[:nsh_rows] = h_bits[r0:r0 + nsh_rows]

        # h prepacked to SBUF-native [partition, tile, free]; x row-major;
        # xt = host-pretransposed X^T halves [2, 128, NSH].
        hp = np.ascontiguousarray(
            h_k.reshape(NT, 128, 128).transpose(1, 0, 2))
        xt = np.ascontiguousarray(
            x_k[:, 0:256].T.reshape(2, 128, NSH))
        in_maps.append(
            {"x": x_k, "xt": xt, "h": hp, "wt": wt, "pv": pv, "gb": gb})
    return in_maps


def kernel(node_embeddings, H, W, b, v, q, ln_gamma, ln_beta):
    global LAST_EXEC_NS, LAST_RESULTS, LN_IDENTITY

    LN_IDENTITY = bool(
        np.all(np.asarray(ln_gamma) == 1.0)
        and np.all(np.asarray(ln_beta) == 0.0))
    in_maps = prepare_in_maps(
        node_embeddings, H, W, b, v, q, ln_gamma, ln_beta)
    nc = _get_nc()
    res = run_bass_kernel_spmd(
        nc, in_maps, core_ids=list(range(NCORES)), trace=PROFILE
    )
    LAST_EXEC_NS = res.exec_time_ns
    LAST_RESULTS = res
    outs = [
        res.results[k]["out_max"][0].astype(np.float32)
        for k in range(NCORES)
    ]
    return np.maximum.reduce(outs).astype(np.float32)
